# revision 28
# baseline (speedup 1.0000x reference)
"""Trainium2 Bass kernel for nn_Conv2dTB (BN -> ternary quantize -> 3x3 conv
-> beta box-filter scaling), data-parallel over batch on 8 NeuronCores.

Contract: kernel(**inputs) takes the FULL unsharded inputs as numpy arrays and
returns the FULL [16, 256, 56, 56] float32 output. Internally the batch dim is
split 2 images/core; BN batch statistics use an on-device AllReduce so the
normalization matches the reference's full-batch statistics.

v2: the 3x3 conv runs as 1D Winograd F(2,3) along y (2/3 of the direct-conv
matmul columns): 4 u-planes per (img, cin-block) are built on the vector
engine from sign rows, the PE contracts them against host-transformed
weights U = G w (with U2 negated so the u2 plane is d1-d2), and the outputs
Ye = c0+c1+c2, Yo = c1-c2-c3 are combined on the vector engine with the
conv bias and beta map fused in. Weights are fully transformed on the host.
"""

import numpy as np

# Problem shapes (hardcoded per contract).
N, C, H, W = 16, 256, 56, 56
COUT = 256
KS = 3
EPS = 1e-4
N_CORES = 8
NLOC = N // N_CORES  # images per core (2)
CB = C // 128  # channel blocks (2)
COB = COUT // 128  # cout blocks (2)
HW = H * W  # 3136
PH = H + 2  # padded rows (58)
PW = W + 2  # padded cols (58)
NA = H // 2  # winograd row-pair tiles per image (28)
JT = 4  # jg groups (4 x 7 row-pairs)
JA = NA // JT  # row-pairs per group (7)
NW = JA * W  # moving cols per matmul (392)
GPIX = 2 * NW  # output pixels per group (784)
COUNT = float(N * H * W)  # BN reduction count (full batch)

_CACHE = {}


def _build():
    import concourse.tile as tile
    from concourse import bacc, mybir

    f32 = mybir.dt.float32
    f16 = mybir.dt.float16
    AF = mybir.ActivationFunctionType
    ALU = mybir.AluOpType

    nc = bacc.Bacc("TRN2", target_bir_lowering=False, debug=False,
                   num_devices=N_CORES)

    # ---- external I/O ----
    x_d = nc.dram_tensor("x", [NLOC, C, H, W], f32, kind="ExternalInput").ap()
    gamma_d = nc.dram_tensor("bn_gamma", [C], f32, kind="ExternalInput").ap()
    bnbeta_d = nc.dram_tensor("bn_beta", [C], f32, kind="ExternalInput").ap()
    wU_d = nc.dram_tensor("wU", [128, CB, 4, KS, COB, 128], f16,
                          kind="ExternalInput").ap()
    cb_d = nc.dram_tensor("conv_b", [COUT], f32, kind="ExternalInput").ap()
    bb_d = nc.dram_tensor("beta_conv_b", [1], f32, kind="ExternalInput").ap()
    ident56_d = nc.dram_tensor("ident56", [H, H], f32,
                               kind="ExternalInput").ap()
    t3_d = nc.dram_tensor("tridiag", [H, H], f32, kind="ExternalInput").ap()
    cnt_d = nc.dram_tensor("boxcnt", [H, W], f32, kind="ExternalInput").ap()
    out_d = nc.dram_tensor("out", [NLOC, COUT, H, W], f32,
                           kind="ExternalOutput").ap()

    import concourse.bass as bass

    with tile.TileContext(nc) as tc:
        with (
            tc.tile_pool(name="persist", bufs=1) as persist,
            tc.tile_pool(name="sq_p", bufs=1) as sq_p,
            tc.tile_pool(name="xclip", bufs=1) as xclip,
            tc.tile_pool(name="stage", bufs=3) as stage,
            tc.tile_pool(name="cstage", bufs=24) as cstage,
            tc.tile_pool(name="gstage", bufs=2) as gstage,
            tc.tile_pool(name="upool", bufs=6) as upool,
            tc.tile_pool(name="outp", bufs=3) as outp,
            tc.tile_pool(name="betabc", bufs=8) as betabc,
            tc.tile_pool(name="ps_w", bufs=6, space="PSUM") as ps_w,
            tc.tile_pool(name="ps_c", bufs=1, space="PSUM") as ps_c,
            tc.tile_pool(name="ps_m", bufs=1, space="PSUM") as ps_m,
            tc.tile_pool(name="dram", bufs=1, space="DRAM") as dram,
        ):
            # ---------- persistent tiles ----------
            x_sb = persist.tile([128, NLOC, CB, HW], f32)
            wU = persist.tile([128, CB, 4, KS, COB, 128], f16)
            t_pad = persist.tile([128, CB, PH, PW], f16)  # shared both imgs
            c2_sb = persist.tile([128, NLOC, HW], f16)
            cT_grid = persist.tile([H, NLOC, PW], f32)
            gamma_sb = persist.tile([128, CB], f32)
            bnbeta_sb = persist.tile([128, CB], f32)
            convb_cols = persist.tile([128, COB], f32)
            ident56_sb = persist.tile([H, H], f32)
            t3_sb = persist.tile([H, H], f32)
            cnt_sb = persist.tile([H, W], f32)
            bb56 = persist.tile([H, 1], f32)
            ones_c = persist.tile([128, 1], f16)
            den56 = persist.tile([H, W], f32)
            invden = persist.tile([H, W], f32)
            # BN stats
            stats = persist.tile([128, CB, NLOC, 2, 2], f32)
            partial = persist.tile([128, 2, CB], f32)  # [kind(sum,sq), cb]
            allred = persist.tile([128, 2, CB], f32)
            meanex = persist.tile([128, 2, CB], f32)
            var_t = persist.tile([128, CB], f32)
            rstd_t = persist.tile([128, CB], f32)
            scale = persist.tile([128, CB], f32)
            shift = persist.tile([128, CB], f32)

            # ---------- 0. no-dep memsets (vector, runs during first DMA) ---
            nc.vector.memset(ones_c[:], 1.0)
            for cb in range(CB):
                nc.vector.memset(t_pad[:, cb, 0, :], 0.0)
                nc.vector.memset(t_pad[:, cb, PH - 1, :], 0.0)
                nc.vector.memset(t_pad[:, cb, 1:PH - 1, 0], 0.0)
                nc.vector.memset(t_pad[:, cb, 1:PH - 1, PW - 1], 0.0)
            for img in range(NLOC):
                nc.vector.memset(cT_grid[:, img, 0:1], 0.0)
                nc.vector.memset(cT_grid[:, img, PW - 1:PW], 0.0)

            # ---------- 1. x DMA (sync queue) + stats interleaved ----------
            for cb in range(CB):
                for img in range(NLOC):
                    xv = x_d[img].rearrange("(cb p) h w -> cb p (h w)", p=128)
                    for a in range(2):
                        sl = slice(a * (HW // 2), (a + 1) * (HW // 2))
                        nc.sync.dma_start(out=x_sb[:, img, cb, sl],
                                          in_=xv[cb][:, sl])
                        nc.vector.reduce_sum(stats[:, cb, img, a, 0:1],
                                             x_sb[:, img, cb, sl],
                                             axis=mybir.AxisListType.X)
                        sq = sq_p.tile([128, HW // 2], f32, tag="sq")
                        nc.scalar.activation(
                            sq[:], x_sb[:, img, cb, sl], AF.Square,
                            accum_out=stats[:, cb, img, a, 1:2])

            # combine partials: partial[:, k, cb] = sum over (img, half)
            for cb in range(CB):
                for img in range(NLOC):
                    nc.vector.tensor_add(stats[:, cb, img, 0, :],
                                         stats[:, cb, img, 0, :],
                                         stats[:, cb, img, 1, :])
                nc.vector.tensor_add(partial[:, :, cb],
                                     stats[:, cb, 0, 0, :],
                                     stats[:, cb, 1, 0, :])

            # ---------- 2. AllReduce ASAP (gpsimd queue is empty) ----------
            bounce_in = dram.tile([128, 2 * CB], f32)
            bounce_out = dram.tile([128, 2 * CB], f32)
            nc.sync.dma_start(out=bounce_in[:],
                              in_=partial[:].rearrange("p a b -> p (a b)"))
            nc.gpsimd.collective_compute(
                "AllReduce", mybir.AluOpType.add,
                replica_groups=[list(range(N_CORES))],
                ins=[bounce_in.opt()], outs=[bounce_out.opt()],
            )
            nc.sync.dma_start(out=allred[:].rearrange("p a b -> p (a b)"),
                              in_=bounce_out[:])

            # ---------- 3. weights + consts DMA (off the critical queues) ---
            for cb in range(CB):
                nc.scalar.dma_start(out=wU[:, cb], in_=wU_d[:, cb])
            nc.scalar.dma_start(out=gamma_sb[:],
                                in_=gamma_d.rearrange("(cb p) -> p cb", p=128))
            nc.scalar.dma_start(out=bnbeta_sb[:],
                                in_=bnbeta_d.rearrange("(cb p) -> p cb",
                                                       p=128))
            nc.scalar.dma_start(out=convb_cols[:],
                                in_=cb_d.rearrange("(cob p) -> p cob", p=128))
            nc.scalar.dma_start(out=ident56_sb[:], in_=ident56_d[:])
            nc.scalar.dma_start(out=t3_sb[:], in_=t3_d[:])
            nc.scalar.dma_start(out=cnt_sb[:], in_=cnt_d[:])
            bbsrc = bb_d[0:1]
            nc.scalar.dma_start(
                out=bb56[:],
                in_=bass.AP(tensor=bbsrc.tensor, offset=bbsrc.offset,
                            ap=[[0, H], [1, 1]]),
            )

            # ---------- 4. scale/shift from allreduced stats ----------
            nc.vector.tensor_scalar_mul(
                meanex[:].rearrange("p a b -> p (a b)"),
                allred[:].rearrange("p a b -> p (a b)"), 1.0 / COUNT)
            mean_t = meanex[:, 0, :]
            ex2_t = meanex[:, 1, :]
            nc.vector.tensor_mul(var_t[:], mean_t, mean_t)
            nc.vector.tensor_sub(var_t[:], ex2_t, var_t[:])
            nc.vector.tensor_scalar_add(var_t[:], var_t[:], EPS)
            nc.vector.reciprocal(var_t[:], var_t[:])
            nc.scalar.activation(rstd_t[:], var_t[:], AF.Sqrt)
            nc.vector.tensor_mul(scale[:], rstd_t[:], gamma_sb[:])
            nc.vector.tensor_mul(shift[:], mean_t, scale[:])
            nc.vector.tensor_sub(shift[:], bnbeta_sb[:], shift[:])

            # 1 / (256 * boxcount + beta_conv_b) for the beta denominator
            nc.vector.tensor_scalar(den56[:], cnt_sb[:], 256.0, bb56[:],
                                    ALU.mult, ALU.add)
            nc.vector.reciprocal(invden[:], den56[:])

            # ---------- helpers ----------
            # uneven halves: h0 must cover padded rows 1..29 so jg0+jg1's
            # u-planes (rows up to 2*13+3=29) only depend on h0
            sign_bounds = [(0, 29), (29, H)]

            def signs_half(img, half):
                # scalar: t = sign(x*scale + shift), one height-half both cbs
                b0, b1 = sign_bounds[half]
                rs = slice(b0, b1)
                prs = slice(1 + b0, 1 + b1)
                for cb in range(CB):
                    tv = t_pad[:, cb, prs, 1:PW - 1]
                    nc.scalar.activation(
                        tv,
                        x_sb[:, img, cb, :].rearrange(
                            "p (h w) -> p h w", w=W)[:, rs, :],
                        AF.Sign, bias=shift[:, cb:cb + 1],
                        scale=scale[:, cb:cb + 1])

            u_tiles = {}

            def vtrans(img, jg):
                # vector: u-planes for row-pairs a0..a0+6 (u-major so the PE
                # can start on u0 while u1.. are still being built)
                a0 = JA * jg
                ut = upool.tile([128, CB, 4, JA, PW], f16, tag="u")
                u_tiles[(img, jg)] = ut
                for u in range(4):
                    for cb in range(CB):
                        ev = t_pad[:, cb].rearrange(
                            "p (a two) w -> p a two w", two=2)
                        od = ev[:, :, 1, :]
                        ev = ev[:, :, 0, :]
                        dst = ut[:, cb, u, :, :]
                        if u == 0:
                            nc.vector.tensor_sub(dst, ev[:, a0:a0 + JA, :],
                                                 ev[:, a0 + 1:a0 + JA + 1, :])
                        elif u == 1:
                            nc.vector.tensor_add(dst, od[:, a0:a0 + JA, :],
                                                 ev[:, a0 + 1:a0 + JA + 1, :])
                        elif u == 2:
                            nc.vector.tensor_sub(dst, od[:, a0:a0 + JA, :],
                                                 ev[:, a0 + 1:a0 + JA + 1, :])
                        else:
                            nc.vector.tensor_sub(dst, od[:, a0:a0 + JA, :],
                                                 od[:, a0 + 1:a0 + JA + 1, :])

            def clip_path(img):
                # c2 = sum_cb min(|x*scale+shift|, 1) (scalar Abs, vector min)
                xts = []
                for cb in range(CB):
                    xt = xclip.tile([128, HW], f16, tag=f"xt{cb}")
                    nc.scalar.activation(xt[:], x_sb[:, img, cb, :], AF.Abs,
                                         bias=shift[:, cb:cb + 1],
                                         scale=scale[:, cb:cb + 1])
                    nc.vector.tensor_scalar_min(xt[:], xt[:], 1.0)
                    xts.append(xt)
                nc.vector.tensor_add(c2_sb[:, img, :], xts[0][:], xts[1][:])

            bflat_ds = [dram.tile([H, W], f16, tag=f"bflat{i}",
                                  name=f"bflat{i}")
                        for i in range(NLOC)]
            ov = out_d.rearrange("n (cob p) h w -> n cob p (h w)", p=128)

            py_tiles = {}
            cstage_tiles = {}
            bbc_tiles = {}

            def conv_mms(img, jg, cob, staged):
                ut = u_tiles[(img, jg)]
                for u in range(4):
                    py = ps_w.tile([128, NW], f32)
                    py_tiles[(img, jg, cob, u)] = py
                    first = True
                    for cbk in range(CB):
                        for kx in range(KS):
                            rhs = ut[:, cbk, u, :, kx:kx + W]
                            nc.tensor.matmul(
                                py[:], wU[:, cbk, u, kx, cob, :], rhs,
                                start=first,
                                stop=(cbk == CB - 1 and kx == KS - 1))
                            first = False
                    if staged:
                        cs = cstage.tile([128, NW], f16, tag="cs")
                        # vector copy: keeps the scalar queue free for the
                        # sign/abs chains that gate the beta path
                        nc.vector.tensor_copy(cs[:], py[:])
                        cstage_tiles[(img, jg, cob, u)] = cs

            def bbc_load(img, jg):
                # gpsimd queue: FIFO-ordered after this image's bflat write.
                # rows 14jg..14jg+13 contiguous; [a, parity, w] layout maps
                # to row 14jg + 2a + par directly
                bbc = betabc.tile([128, JA, 2, W], f16, tag="bbc")
                src = bflat_ds[img][2 * JA * jg]
                nc.gpsimd.dma_start(
                    out=bbc[:],
                    in_=bass.AP(tensor=src.tensor, offset=src.offset,
                                ap=[[0, 128], [1, GPIX]]))
                bbc_tiles[(img, jg)] = bbc

            def conv_out(img, jg, cob):
                bbc = bbc_tiles[(img, jg)]
                cvb = convb_cols[:, cob:cob + 1]
                key = (img, jg, cob)
                staged = key + (0,) in cstage_tiles

                def cget(u):
                    return (cstage_tiles[key + (u,)][:] if staged
                            else py_tiles[key + (u,)][:])

                if staged:
                    c1, c2k = cget(1), cget(2)
                else:
                    c1 = cstage.tile([128, NW], f16, tag="cs")
                    nc.scalar.copy(c1[:], py_tiles[key + (1,)][:])
                    c2k = cstage.tile([128, NW], f16, tag="cs")
                    nc.scalar.copy(c2k[:], py_tiles[key + (2,)][:])
                    c1, c2k = c1[:], c2k[:]
                osb = outp.tile([128, JA, 2, W], f32, tag="osb")
                # Ye = c0 + c1 + c2 (+bias), then * beta
                ta = gstage.tile([128, NW], f16, tag="ta")
                nc.vector.scalar_tensor_tensor(ta[:], cget(0), cvb, c1,
                                               ALU.add, ALU.add)
                ye = gstage.tile([128, NW], f16, tag="ye")
                nc.vector.tensor_add(ye[:], ta[:], c2k)
                nc.vector.tensor_mul(
                    osb[:, :, 0, :],
                    ye[:].rearrange("p (a w) -> p a w", w=W), bbc[:, :, 0, :])
                # Yo = c1 - c2 - c3 (+bias), then * beta
                tc_ = gstage.tile([128, NW], f16, tag="tc")
                nc.vector.scalar_tensor_tensor(tc_[:], cget(3), cvb, c2k,
                                               ALU.subtract, ALU.add)
                td = gstage.tile([128, NW], f16, tag="td")
                nc.vector.tensor_sub(td[:], c1, tc_[:])
                nc.vector.tensor_mul(
                    osb[:, :, 1, :],
                    td[:].rearrange("p (a w) -> p a w", w=W), bbc[:, :, 1, :])
                nc.sync.dma_start(
                    out=ov[img, cob][:, jg * GPIX:(jg + 1) * GPIX],
                    in_=osb[:].rearrange("p a two w -> p (a two w)"))

            def beta_path(img):
                # channel sums -> cT_grid[x, 1+y] (PE, one row per matmul)
                for rt in range(7):
                    pct = ps_c.tile([H, 8], f32)
                    for r in range(8):
                        y = rt * 8 + r
                        nc.tensor.matmul(
                            pct[:, r:r + 1],
                            c2_sb[:, img, y * W:(y + 1) * W],
                            ones_c[:], start=True, stop=True)
                    nc.scalar.copy(
                        cT_grid[:, img, 1 + rt * 8:1 + (rt + 1) * 8],
                        pct[:])
                # box over y (free dim), then over x via tridiagonal matmul
                hsumT = stage.tile([H, W], f32, tag="hsumT")
                cg = cT_grid[:, img, :]
                nc.gpsimd.tensor_add(hsumT[:], cg[:, 0:W], cg[:, 1:W + 1])
                nc.gpsimd.tensor_add(hsumT[:], hsumT[:], cg[:, 2:W + 2])
                pbT = ps_m.tile([H, W], f32, tag="psm")
                nc.tensor.matmul(pbT[:], t3_sb[:], hsumT[:], start=True,
                                 stop=True)
                bmapT = stage.tile([H, W], f32, tag="bmapT")
                nc.scalar.activation(bmapT[:], pbT[:], AF.Identity,
                                     bias=bb56[:])
                nc.gpsimd.tensor_mul(bmapT[:], bmapT[:], invden[:])
                pbm = ps_m.tile([H, W], f32, tag="psm")
                nc.tensor.transpose(pbm[:], bmapT[:], ident56_sb[:])
                bmap = stage.tile([H, W], f16, tag="bmap")
                nc.scalar.copy(bmap[:], pbm[:])
                nc.gpsimd.dma_start(out=bflat_ds[img][:], in_=bmap[:])

            # ---------- 5+6. per-image pipeline ----------
            signs_half(0, 0)  # covers jg0+jg1 (padded rows 1..29)
            for jg in (0, 1):
                vtrans(0, jg)
            conv_mms(0, 0, 0, staged=True)
            conv_mms(0, 0, 1, staged=True)
            signs_half(0, 1)
            for jg in (2, 3):
                vtrans(0, jg)
            conv_mms(0, 1, 0, staged=True)
            conv_mms(0, 1, 1, staged=True)
            clip_path(0)
            # img1 sign/clip early: scalar is free once abs(i0) is done
            signs_half(1, 0)
            for jg in (0, 1):
                vtrans(1, jg)
            signs_half(1, 1)
            for jg in (2, 3):
                vtrans(1, jg)
            clip_path(1)
            beta_path(0)
            for jg in range(JT):
                bbc_load(0, jg)
            conv_mms(0, 2, 0, staged=False)
            conv_mms(0, 2, 1, staged=False)
            conv_out(0, 2, 0)
            conv_out(0, 2, 1)
            conv_mms(0, 3, 0, staged=False)
            conv_mms(0, 3, 1, staged=False)
            conv_out(0, 3, 0)
            conv_out(0, 3, 1)
            # staged groups drained early via copies; combines can run late
            conv_out(0, 0, 0)
            conv_out(0, 0, 1)
            conv_out(0, 1, 0)
            conv_out(0, 1, 1)
            conv_mms(1, 0, 0, staged=False)
            conv_mms(1, 0, 1, staged=False)
            beta_path(1)
            for jg in range(JT):
                bbc_load(1, jg)
            conv_out(1, 0, 0)
            conv_out(1, 0, 1)
            for jg in range(1, JT):
                conv_mms(1, jg, 0, staged=False)
                conv_mms(1, jg, 1, staged=False)
                conv_out(1, jg, 0)
                conv_out(1, jg, 1)

    nc.compile()
    return nc


def _consts():
    ident56 = np.eye(H, dtype=np.float32)
    t3 = np.zeros((H, H), dtype=np.float32)
    for i in range(H):
        for j in range(max(0, i - 1), min(H, i + 2)):
            t3[j, i] = 1.0
    r = np.minimum(np.arange(H), H - 1 - np.arange(H))
    edge = (r >= 1).astype(np.float32) + 2.0  # 2 on border rows, 3 inside
    cnt = np.outer(edge, edge).astype(np.float32)  # valid taps: 4/6/9
    return ident56, t3, cnt


def _prep_inputs(inputs):
    """Full inputs -> per-core in_maps (host-side shard + weight prep)."""
    x = np.ascontiguousarray(inputs["x"], dtype=np.float32)
    w = np.asarray(inputs["conv_w"], dtype=np.float32)  # [COUT, C, 3, 3]
    w0, w1, w2 = w[:, :, 0, :], w[:, :, 1, :], w[:, :, 2, :]
    # F(2,3) weight transform along ky; U2 negated to pair with u2 = d1-d2
    w4 = np.stack([w0, (w0 + w1 + w2) * 0.5, -(w0 - w1 + w2) * 0.5, w2])
    # wU[p, cbk, u, kx, cob, m] = w4[u, cob*128+m, cbk*128+p, kx]
    wU = np.ascontiguousarray(
        w4.reshape(4, COB, 128, CB, 128, KS).transpose(4, 3, 0, 5, 1, 2)
    ).astype(np.float16)
    ident56, t3, cnt = _consts()
    shared = {
        "bn_gamma": np.ascontiguousarray(inputs["bn_gamma"], np.float32),
        "bn_beta": np.ascontiguousarray(inputs["bn_beta"], np.float32),
        "wU": wU,
        "conv_b": np.ascontiguousarray(inputs["conv_b"], np.float32),
        "beta_conv_b": np.ascontiguousarray(inputs["beta_conv_b"],
                                            np.float32),
        "ident56": ident56, "tridiag": t3, "boxcnt": cnt,
    }
    return [
        {"x": np.ascontiguousarray(x[i * NLOC:(i + 1) * NLOC]), **shared}
        for i in range(N_CORES)
    ]


def kernel(**inputs):
    from concourse.bass_utils import run_bass_kernel_spmd

    if "nc" not in _CACHE:
        _CACHE["nc"] = _build()
    nc = _CACHE["nc"]

    in_maps = _prep_inputs(inputs)
    res = run_bass_kernel_spmd(nc, in_maps, list(range(N_CORES)))
    out = np.concatenate([res.results[i]["out"] for i in range(N_CORES)],
                         axis=0)
    return out.astype(np.float32)
